# revision 21
# baseline (speedup 1.0000x reference)
"""Fused MoE + per-expert LoRA, expert-parallel across 8 TRN2 NeuronCores.

Strategy (sharding hint: expert-parallel):
  - Host dispatches the T*K routed (token, expert) pairs: core e gets the
    tokens routed to expert e, gathered + transposed to [H, C] (C = padded
    max per-expert count), plus expert e's w13/w2 and the 4 LoRA adapters'
    A/B for expert e, all pre-transposed on host into the exact SBUF
    layouts the kernel consumes (contiguous DMAs).
  - LoRA is fused into the base GEMMs: the 4 adapters' A matrices are
    concatenated to [64, H] so one extra K=128-tile matmul chain produces
    all mid-products; a [64, C] mask (scaling[l] where the pair's adapter
    == l, else 0) selects the right adapter per column; the 4 B matrices
    concatenated to [64, N] accumulate into the same PSUM tile as the base
    GEMM (one extra K=64 matmul per output tile).
  - Routing weight is folded into the activation (act = silu(gate)*up*w),
    which makes both the base down-GEMM and the down-LoRA delta carry it.
  - Host scatter-adds each core's [H, C] output back over the top_k axis.
  - Matmuls run in float32r (full-rate fp32 streaming, ~1e-4 rel err).
"""

import numpy as np
from contextlib import ExitStack

import concourse.bass as bass
import concourse.tile as tile
from concourse import bacc, mybir
from concourse.bass_utils import run_bass_kernel_spmd

T, H, I, E, K, L, R = 1024, 1024, 1024, 8, 2, 4, 16
N = 2 * I
P = 128
KH = H // P    # k-tiles over H (gate_up contraction)
KI = I // P    # k-tiles over I (down contraction)
NT = N // P    # n-tiles of gate_up output
HT = H // P    # h-tiles of down output
LR = L * R     # concatenated lora rank

# w13 SBUF slot s holds n-tile W13_PERM[s]: gate/up tiles interleaved so the
# compute loop consumes slots sequentially (pair j = slots 2j, 2j+1)
W13_PERM = [t for j in range(NT // 2) for t in (j, j + NT // 2)]

_CACHE: dict = {}

# f16 halves HBM traffic vs f32 (w13+w2 dominate); matmul streams at the
# same 1 col/cycle either way, so this moves the kernel from DMA-bound
# (~46us of weight DMA) to PE-bound (~29us). randn-scale data sits well
# inside f16 range; rel err ~1e-3 << the 2e-2 gate.
MODE = "f16"
ACT_FN = None  # debug hook: sim_check overrides (CoreSim lacks Silu)


def _round_up(x, m):
    return ((x + m - 1) // m) * m


def _np_dt(mode):
    if mode == "bf16":
        import ml_dtypes
        return np.dtype(ml_dtypes.bfloat16)
    if mode == "f16":
        return np.dtype(np.float16)
    return np.dtype(np.float32)


def _mm_dt(mode):
    return {"f32": mybir.dt.float32,
            "f32r": mybir.dt.float32r,
            "bf16": mybir.dt.bfloat16,
            "f16": mybir.dt.float16}[mode]


def _prep_in_maps(hidden_states, topk_weights, w13, w2, gate_up_lora_a,
                  gate_up_lora_b, down_lora_a, down_lora_b, scalings,
                  topk_ids, lora_indices, mode=None):
    """Host-side dispatch: returns (in_maps, idx_per_expert, tok, C)."""
    mode = mode or MODE
    ndt = _np_dt(mode)
    hidden_states = np.asarray(hidden_states, dtype=np.float32)
    topk_weights = np.asarray(topk_weights, dtype=np.float32)
    w13 = np.asarray(w13, dtype=np.float32)
    w2 = np.asarray(w2, dtype=np.float32)
    gua = np.asarray(gate_up_lora_a, dtype=np.float32)
    gub = np.asarray(gate_up_lora_b, dtype=np.float32)
    dla = np.asarray(down_lora_a, dtype=np.float32)
    dlb = np.asarray(down_lora_b, dtype=np.float32)
    scalings = np.asarray(scalings, dtype=np.float32)
    topk_ids = np.asarray(topk_ids)
    lora_indices = np.asarray(lora_indices)

    e_flat = topk_ids.reshape(-1).astype(np.int64)
    l_flat = np.repeat(lora_indices, K).astype(np.int64)
    w_flat = topk_weights.reshape(-1).astype(np.float32)
    tok = np.arange(T * K) // K

    idx_per = [np.nonzero(e_flat == e)[0] for e in range(E)]
    maxc = max(1, max(len(ix) for ix in idx_per))
    C = max(128, _round_up(maxc, 4))

    in_maps = []
    for e in range(E):
        ix = idx_per[e]
        cnt = len(ix)

        xg = np.zeros((C, H), np.float32)
        xg[:cnt] = hidden_states[tok[ix]]
        xt = np.ascontiguousarray(
            xg.T.reshape(KH, P, C).transpose(1, 0, 2).reshape(P, KH * C)).astype(ndt)

        w13t = w13[e].T  # [H, N]
        # slot-major layout: slot s (gate/up interleaved via W13_PERM) is a
        # contiguous [P, KH*P] span -> per-slot DMAs are contiguous
        w13_tiles = w13t.reshape(KH, P, NT, P)[:, :, W13_PERM]
        w13_hbm = np.ascontiguousarray(
            w13_tiles.transpose(1, 2, 0, 3).reshape(P, KH * N)).astype(ndt)

        w2t = w2[e].T  # [I, H]
        w2_hbm = np.ascontiguousarray(
            w2t.reshape(KI, P, HT, P)
            .transpose(1, 2, 0, 3).reshape(P, KI * H)).astype(ndt)

        ag = np.ascontiguousarray(
            gua[:, e].reshape(LR, H).T.reshape(KH, P, LR)
            .transpose(1, 0, 2).reshape(P, KH * LR)).astype(ndt)
        ad = np.ascontiguousarray(
            dla[:, e].reshape(LR, I).T.reshape(KI, P, LR)
            .transpose(1, 0, 2).reshape(P, KI * LR)).astype(ndt)
        bg = np.ascontiguousarray(
            gub[:, e].transpose(0, 2, 1).reshape(LR, N)).astype(ndt)
        bd = np.ascontiguousarray(
            dlb[:, e].transpose(0, 2, 1).reshape(LR, H)).astype(ndt)

        msk = np.zeros((LR, C), np.float32)
        if cnt:
            lv = l_flat[ix]
            m_small = (lv[None, :] == np.arange(L)[:, None]) * scalings[:, None]
            msk[:, :cnt] = np.repeat(m_small.astype(np.float32), R, axis=0)

        wv = np.zeros((P, C), np.float32)
        if cnt:
            wv[:, :cnt] = w_flat[ix][None, :]

        in_maps.append({
            "xt": xt, "w13t": w13_hbm, "w2t": w2_hbm,
            "agt": ag, "adt": ad, "bgt": bg, "bdt": bd,
            "msk": msk, "wv": wv,
        })
    return in_maps, idx_per, tok, C


def _combine(results, idx_per, tok, C):
    out = np.zeros((T, H), np.float32)
    for e in range(E):
        ix = idx_per[e]
        cnt = len(ix)
        if cnt == 0:
            continue
        outt = results[e]["outt"].reshape(P, HT, C).transpose(1, 0, 2).reshape(H, C)
        np.add.at(out, tok[ix], outt[:, :cnt].T)
    return out


def _build(C, mode=None, repeat=1, loop_reps=0, body="full"):
    """Trace + compile the per-core bass program for padded count C.

    loop_reps > 0 wraps the body in a device-side For_i loop (timing only).
    body: "full" | "dma" (loads/stores only, no compute) | "compute"
    (loads hoisted out of the timing loop) - diagnostics only.
    """
    mode = mode or MODE
    f32 = mybir.dt.float32
    mdt = _mm_dt(mode)
    nc = bacc.Bacc("TRN2", target_bir_lowering=False, debug=False, num_devices=E)

    xt_d = nc.declare_dram_parameter("xt", [P, KH * C], mdt, isOutput=False)
    w13_d = nc.declare_dram_parameter("w13t", [P, KH * N], mdt, isOutput=False)
    w2_d = nc.declare_dram_parameter("w2t", [P, KI * H], mdt, isOutput=False)
    ag_d = nc.declare_dram_parameter("agt", [P, KH * LR], mdt, isOutput=False)
    ad_d = nc.declare_dram_parameter("adt", [P, KI * LR], mdt, isOutput=False)
    bg_d = nc.declare_dram_parameter("bgt", [LR, N], mdt, isOutput=False)
    bd_d = nc.declare_dram_parameter("bdt", [LR, H], mdt, isOutput=False)
    msk_d = nc.declare_dram_parameter("msk", [LR, C], f32, isOutput=False)
    wv_d = nc.declare_dram_parameter("wv", [P, C], f32, isOutput=False)
    odt = f32 if mybir.dt.size(mdt) == 4 else mdt
    out_d = nc.declare_dram_parameter("outt", [P, HT * C], odt, isOutput=True)

    # column blocks of at most 512 (PSUM free-dim limit for fp32)
    n_blk = (C + 511) // 512
    step = (C + n_blk - 1) // n_blk
    blks = [(b * step, min(C, (b + 1) * step)) for b in range(n_blk)]

    silu_fn = ACT_FN or mybir.ActivationFunctionType.Silu

    with tile.TileContext(nc) as tc:
        with ExitStack() as ctx:
            static = ctx.enter_context(tc.tile_pool(name="static", bufs=1))
            work = ctx.enter_context(tc.tile_pool(name="work", bufs=4))
            ptmp = ctx.enter_context(tc.tile_pool(name="ptmp", bufs=1, space="PSUM"))
            pc1g_pool = ctx.enter_context(tc.tile_pool(name="pc1g", bufs=3, space="PSUM"))
            pc1u_pool = ctx.enter_context(tc.tile_pool(name="pc1u", bufs=2, space="PSUM"))
            pc3 = ctx.enter_context(tc.tile_pool(name="pc3", bufs=2, space="PSUM"))

            # double-buffer input tiles for 2-byte modes so a loop/repeat
            # iteration's DMAs overlap the previous iteration's compute
            # (f32 tiles are too big to double-buffer in SBUF)
            sb = 2 if mybir.dt.size(mdt) == 2 else 1

            def make_tiles():
                t = {}
                t["xt"] = static.tile([P, KH * C], mdt, tag="xt", name="xt_sb", bufs=sb)
                t["w13"] = static.tile([P, KH * N], mdt, tag="w13", name="w13_sb", bufs=sb)
                t["w2"] = static.tile([P, KI * H], mdt, tag="w2", name="w2_sb", bufs=sb)
                t["ag"] = static.tile([P, KH * LR], mdt, tag="ag", name="ag_sb", bufs=sb)
                t["ad"] = static.tile([P, KI * LR], mdt, tag="ad", name="ad_sb", bufs=sb)
                t["bg"] = static.tile([LR, N], mdt, tag="bg", name="bg_sb", bufs=sb)
                t["bd"] = static.tile([LR, H], mdt, tag="bd", name="bd_sb", bufs=sb)
                t["msk"] = static.tile([LR, C], f32, tag="msk", name="msk_sb", bufs=sb)
                t["wv"] = static.tile([P, C], f32, tag="wv", name="wv_sb", bufs=sb)
                t["act"] = static.tile([P, KI * C], mdt, tag="act", name="act_sb")
                t["out"] = static.tile([P, HT * C], odt, tag="out", name="out_sb")
                t["xlg"] = static.tile([LR, C], mdt, tag="xlg", name="xlg_sb")
                t["xld"] = static.tile([LR, C], mdt, tag="xld", name="xld_sb")
                return t

            SW13 = KH * P   # free-dim cols per w13 slot (slot-major)
            SW2 = KI * P    # free-dim cols per w2 h-tile

            def emit_loads(t):
                # scalar ring: only the first few gate-critical loads (its
                # triggers retire before the first silu needs the engine).
                # sync ring: everything else in consumption order — once the
                # scalar ring drains (~t+13) a single HWDGE ring sustains
                # ~390 GB/s, so the tail (w2 etc) is not bandwidth-starved.
                sy, sc = nc.sync, nc.scalar

                def xk(ring, k0, k1):
                    ring.dma_start(t["xt"][:, k0 * C:k1 * C],
                                   xt_d[:, k0 * C:k1 * C])

                def ws(ring, s0, s1):
                    ring.dma_start(t["w13"][:, s0 * SW13:s1 * SW13],
                                   w13_d[:, s0 * SW13:s1 * SW13])

                xk(sy, 0, 1)
                sc.dma_start(t["w13"][:, :SW13 // 2], w13_d[:, :SW13 // 2])
                sc.dma_start(t["w13"][:, SW13 // 2:SW13], w13_d[:, SW13 // 2:SW13])
                xk(sy, 1, 5)
                ws(sc, 2, 3)
                xk(sy, 5, 8)
                ws(sc, 3, 4)
                ws(sy, 1, 2)
                sy.dma_start(t["ag"][:], ag_d[:])
                sy.dma_start(t["msk"][:], msk_d[:])
                sy.dma_start(t["bg"][:], bg_d[:])
                for s in range(4, NT, 2):
                    ws(sy, s, s + 2)
                sy.dma_start(t["ad"][:], ad_d[:])
                sy.dma_start(t["wv"][:], wv_d[:])
                sy.dma_start(t["bd"][:], bd_d[:])
                sy.dma_start(t["w2"][:, :HT // 2 * SW2], w2_d[:, :HT // 2 * SW2])
                sy.dma_start(t["w2"][:, HT // 2 * SW2:], w2_d[:, HT // 2 * SW2:])

            def emit_compute(t):
                # preload the ACT silu table at t~0 (otherwise a lazy 1.3us
                # ACT_TABLE_LOAD lands right before the first real silu and
                # stalls the pc1 PSUM recycle chain)
                warm = work.tile([P, 8], f32, tag="actwarm")
                nc.gpsimd.memset(warm[:], 0.0)
                nc.scalar.activation(warm[:], warm[:], silu_fn)

                # pre-warm the PE while the first DMAs are in flight: dummy
                # matmuls on a zeroed tile keep the HAM activity window busy
                # so the clock is at 2.4 GHz (not 1.2) when real data lands
                wcw = min(blks[0][1] - blks[0][0], 384 - P)
                wsrc = work.tile([P, 384], mdt, tag="mmwarm")
                nc.gpsimd.memset(wsrc[:], 0.0)
                wp = pc3.tile([P, wcw], f32, tag="c3")

                def fill(n):
                    for _ in range(n):
                        nc.tensor.matmul(wp[:], wsrc[:, :P],
                                         wsrc[:, P:P + wcw],
                                         start=True, stop=True)

                fill(17)

                for (c0, c1) in blks:
                    cw = c1 - c0

                    def xts(kt):
                        return t["xt"][:, kt * C + c0: kt * C + c1]

                    def acts(kt):
                        return t["act"][:, kt * C + c0: kt * C + c1]

                    def base13(c1t, slot):
                        for kt in range(KH):
                            off = slot * SW13 + kt * P
                            nc.tensor.matmul(
                                c1t[:], t["w13"][:, off:off + P], xts(kt),
                                start=(kt == 0), stop=False)

                    def bapply(c1t, jn):
                        nc.tensor.matmul(
                            c1t[:], t["bg"][:, jn * P:(jn + 1) * P],
                            t["xlg"][:, c0:c1], start=False, stop=True)

                    def drain(j, c1g, c1u):
                        silu_t = work.tile([P, cw], f32, tag="silu")
                        nc.scalar.activation(silu_t[:], c1g[:], silu_fn)
                        nc.vector.tensor_mul(acts(j), c1u[:], silu_t[:])

                    # pairs 0,1: base chains first so PE starts on the first
                    # w13 slot + xt chunk; the lora mid-product fills the
                    # window while slot2/slot3 are still in flight
                    held = []
                    c1g = pc1g_pool.tile([P, cw], f32, tag="c1g")
                    base13(c1g, 0)
                    fill(5)
                    c1u = pc1u_pool.tile([P, cw], f32, tag="c1u")
                    base13(c1u, 1)
                    held.append((0, c1g, c1u))
                    fill(8)

                    # gate_up lora mid-product
                    tmp_g = ptmp.tile([LR, cw], f32, tag="tmp")
                    for kt in range(KH):
                        nc.tensor.matmul(
                            tmp_g[:], t["ag"][:, kt * LR:(kt + 1) * LR], xts(kt),
                            start=(kt == 0), stop=(kt == KH - 1))

                    fill(4)
                    c1g = pc1g_pool.tile([P, cw], f32, tag="c1g")
                    base13(c1g, 2)
                    fill(6)
                    c1u = pc1u_pool.tile([P, cw], f32, tag="c1u")
                    base13(c1u, 3)
                    held.append((1, c1g, c1u))

                    nc.vector.tensor_mul(t["xlg"][:, c0:c1], tmp_g[:],
                                         t["msk"][:, c0:c1])

                    for (j, c1g, c1u) in held:
                        bapply(c1g, j)
                        bapply(c1u, j + KI)
                        drain(j, c1g, c1u)

                    for j in range(2, KI):
                        c1g = pc1g_pool.tile([P, cw], f32, tag="c1g")
                        base13(c1g, 2 * j)
                        bapply(c1g, j)
                        c1u = pc1u_pool.tile([P, cw], f32, tag="c1u")
                        base13(c1u, 2 * j + 1)
                        bapply(c1u, j + KI)
                        drain(j, c1g, c1u)

                    # down lora mid-product
                    tmp_d = ptmp.tile([LR, cw], f32, tag="tmp")
                    for kt in range(KI):
                        nc.tensor.matmul(
                            tmp_d[:], t["ad"][:, kt * LR:(kt + 1) * LR], acts(kt),
                            start=(kt == 0), stop=(kt == KI - 1))
                    nc.vector.tensor_mul(t["xld"][:, c0:c1], tmp_d[:],
                                         t["msk"][:, c0:c1])

                    # down GEMM + lora; routing weight applied at the output
                    for h in range(HT):
                        c3t = pc3.tile([P, cw], f32, tag="c3")
                        for kt in range(KI):
                            off = h * SW2 + kt * P
                            nc.tensor.matmul(
                                c3t[:], t["w2"][:, off:off + P], acts(kt),
                                start=(kt == 0), stop=False)
                        nc.tensor.matmul(
                            c3t[:], t["bd"][:, h * P:(h + 1) * P],
                            t["xld"][:, c0:c1], start=False, stop=True)
                        nc.vector.tensor_mul(
                            t["out"][:, h * C + c0: h * C + c1], c3t[:],
                            t["wv"][:, c0:c1])
                        # stores ride the scalar HWDGE ring, which is idle
                        # by the time the down phase produces output tiles
                        if c0 == 0 and c1 == C and (h % 2 == 1 or h >= HT - 2):
                            lo = (h - 1) * C if (h % 2 == 1 and h < HT - 1) else h * C
                            nc.scalar.dma_start(
                                out_d[:, lo:(h + 1) * C],
                                t["out"][:, lo:(h + 1) * C])
            def emit_block_stores(t):
                # fallback for multi-block shapes (C > 512): bulk store
                if not (len(blks) == 1 and blks[0] == (0, C)):
                    half = HT * C // 2
                    nc.sync.dma_start(out_d[:, :half], t["out"][:, :half])
                    nc.scalar.dma_start(out_d[:, half:], t["out"][:, half:])

            hoisted = None
            if body == "compute":
                hoisted = make_tiles()
                emit_loads(hoisted)

            loop_ctx = None
            if loop_reps > 0:
                loop_ctx = tc.For_i(
                    0, loop_reps, 1,
                    hint_engines=(mybir.EngineType.PE, mybir.EngineType.DVE,
                                  mybir.EngineType.Activation,
                                  mybir.EngineType.SP))
                loop_ctx.__enter__()

            for _rep in range(repeat):
                t = hoisted if hoisted is not None else make_tiles()
                if body == "full":
                    emit_loads(t)
                    emit_compute(t)
                elif body == "dma":
                    emit_loads(t)
                elif body == "compute":
                    emit_compute(t)
                if body != "dma":
                    emit_block_stores(t)

            if loop_ctx is not None:
                loop_ctx.__exit__(None, None, None)

    nc.compile()
    return nc


def _get_nc(C, mode=None, repeat=1, loop_reps=0, body="full"):
    mode = mode or MODE
    key = (C, mode, repeat, loop_reps, body)
    if key not in _CACHE:
        _CACHE[key] = _build(C, mode, repeat, loop_reps, body)
    return _CACHE[key]


def kernel(hidden_states, topk_weights, w13, w2, gate_up_lora_a,
           gate_up_lora_b, down_lora_a, down_lora_b, scalings,
           topk_ids, lora_indices, mode=None):
    in_maps, idx_per, tok, C = _prep_in_maps(
        hidden_states, topk_weights, w13, w2, gate_up_lora_a,
        gate_up_lora_b, down_lora_a, down_lora_b, scalings,
        topk_ids, lora_indices, mode=mode)
    nc = _get_nc(C, mode)
    res = run_bass_kernel_spmd(nc, in_maps, list(range(E)))
    out = _combine(res.results, idx_per, tok, C)
    return out.astype(np.asarray(hidden_states).dtype)



# revision 22
# speedup vs baseline: 1.0232x; 1.0232x over previous
"""Fused MoE + per-expert LoRA, expert-parallel across 8 TRN2 NeuronCores.

Strategy (sharding hint: expert-parallel):
  - Host dispatches the T*K routed (token, expert) pairs: core e gets the
    tokens routed to expert e, gathered + transposed to [H, C] (C = padded
    max per-expert count), plus expert e's w13/w2 and the 4 LoRA adapters'
    A/B for expert e, all pre-transposed on host into the exact SBUF
    layouts the kernel consumes (contiguous DMAs).
  - LoRA is fused into the base GEMMs: the 4 adapters' A matrices are
    concatenated to [64, H] so one extra K=128-tile matmul chain produces
    all mid-products; a [64, C] mask (scaling[l] where the pair's adapter
    == l, else 0) selects the right adapter per column; the 4 B matrices
    concatenated to [64, N] accumulate into the same PSUM tile as the base
    GEMM (one extra K=64 matmul per output tile).
  - Routing weight is folded into the activation (act = silu(gate)*up*w),
    which makes both the base down-GEMM and the down-LoRA delta carry it.
  - Host scatter-adds each core's [H, C] output back over the top_k axis.
  - Matmuls run in float32r (full-rate fp32 streaming, ~1e-4 rel err).
"""

import numpy as np
from contextlib import ExitStack

import concourse.bass as bass
import concourse.tile as tile
from concourse import bacc, mybir
from concourse.bass_utils import run_bass_kernel_spmd

T, H, I, E, K, L, R = 1024, 1024, 1024, 8, 2, 4, 16
N = 2 * I
P = 128
KH = H // P    # k-tiles over H (gate_up contraction)
KI = I // P    # k-tiles over I (down contraction)
NT = N // P    # n-tiles of gate_up output
HT = H // P    # h-tiles of down output
LR = L * R     # concatenated lora rank

# w13 SBUF slot s holds n-tile W13_PERM[s]: gate/up tiles interleaved so the
# compute loop consumes slots sequentially (pair j = slots 2j, 2j+1)
W13_PERM = [t for j in range(NT // 2) for t in (j, j + NT // 2)]

_CACHE: dict = {}

# f16 halves HBM traffic vs f32 (w13+w2 dominate); matmul streams at the
# same 1 col/cycle either way, so this moves the kernel from DMA-bound
# (~46us of weight DMA) to PE-bound (~29us). randn-scale data sits well
# inside f16 range; rel err ~1e-3 << the 2e-2 gate.
MODE = "f16"
ACT_FN = None  # debug hook: sim_check overrides (CoreSim lacks Silu)


def _round_up(x, m):
    return ((x + m - 1) // m) * m


def _np_dt(mode):
    if mode == "bf16":
        import ml_dtypes
        return np.dtype(ml_dtypes.bfloat16)
    if mode == "f16":
        return np.dtype(np.float16)
    return np.dtype(np.float32)


def _mm_dt(mode):
    return {"f32": mybir.dt.float32,
            "f32r": mybir.dt.float32r,
            "bf16": mybir.dt.bfloat16,
            "f16": mybir.dt.float16}[mode]


def _prep_in_maps(hidden_states, topk_weights, w13, w2, gate_up_lora_a,
                  gate_up_lora_b, down_lora_a, down_lora_b, scalings,
                  topk_ids, lora_indices, mode=None):
    """Host-side dispatch: returns (in_maps, idx_per_expert, tok, C)."""
    mode = mode or MODE
    ndt = _np_dt(mode)
    hidden_states = np.asarray(hidden_states, dtype=np.float32)
    topk_weights = np.asarray(topk_weights, dtype=np.float32)
    w13 = np.asarray(w13, dtype=np.float32)
    w2 = np.asarray(w2, dtype=np.float32)
    gua = np.asarray(gate_up_lora_a, dtype=np.float32)
    gub = np.asarray(gate_up_lora_b, dtype=np.float32)
    dla = np.asarray(down_lora_a, dtype=np.float32)
    dlb = np.asarray(down_lora_b, dtype=np.float32)
    scalings = np.asarray(scalings, dtype=np.float32)
    topk_ids = np.asarray(topk_ids)
    lora_indices = np.asarray(lora_indices)

    e_flat = topk_ids.reshape(-1).astype(np.int64)
    l_flat = np.repeat(lora_indices, K).astype(np.int64)
    w_flat = topk_weights.reshape(-1).astype(np.float32)
    tok = np.arange(T * K) // K

    idx_per = [np.nonzero(e_flat == e)[0] for e in range(E)]
    maxc = max(1, max(len(ix) for ix in idx_per))
    C = max(128, _round_up(maxc, 4))

    in_maps = []
    for e in range(E):
        ix = idx_per[e]
        cnt = len(ix)

        xg = np.zeros((C, H), np.float32)
        xg[:cnt] = hidden_states[tok[ix]]
        xt = np.ascontiguousarray(
            xg.T.reshape(KH, P, C).transpose(1, 0, 2).reshape(P, KH * C)).astype(ndt)

        w13t = w13[e].T  # [H, N]
        # slot-major layout: slot s (gate/up interleaved via W13_PERM) is a
        # contiguous [P, KH*P] span -> per-slot DMAs are contiguous
        w13_tiles = w13t.reshape(KH, P, NT, P)[:, :, W13_PERM]
        w13_hbm = np.ascontiguousarray(
            w13_tiles.transpose(1, 2, 0, 3).reshape(P, KH * N)).astype(ndt)

        w2t = w2[e].T  # [I, H]
        w2_hbm = np.ascontiguousarray(
            w2t.reshape(KI, P, HT, P)
            .transpose(1, 2, 0, 3).reshape(P, KI * H)).astype(ndt)

        ag = np.ascontiguousarray(
            gua[:, e].reshape(LR, H).T.reshape(KH, P, LR)
            .transpose(1, 0, 2).reshape(P, KH * LR)).astype(ndt)
        ad = np.ascontiguousarray(
            dla[:, e].reshape(LR, I).T.reshape(KI, P, LR)
            .transpose(1, 0, 2).reshape(P, KI * LR)).astype(ndt)
        bg = np.ascontiguousarray(
            gub[:, e].transpose(0, 2, 1).reshape(LR, N)).astype(ndt)
        bd = np.ascontiguousarray(
            dlb[:, e].transpose(0, 2, 1).reshape(LR, H)).astype(ndt)

        msk = np.zeros((LR, C), np.float32)
        if cnt:
            lv = l_flat[ix]
            m_small = (lv[None, :] == np.arange(L)[:, None]) * scalings[:, None]
            msk[:, :cnt] = np.repeat(m_small.astype(np.float32), R, axis=0)

        wv = np.zeros((P, C), np.float32)
        if cnt:
            wv[:, :cnt] = w_flat[ix][None, :]

        in_maps.append({
            "xt": xt, "w13t": w13_hbm, "w2t": w2_hbm,
            "agt": ag, "adt": ad, "bgt": bg, "bdt": bd,
            "msk": msk, "wv": wv,
        })
    return in_maps, idx_per, tok, C


def _combine(results, idx_per, tok, C):
    out = np.zeros((T, H), np.float32)
    for e in range(E):
        ix = idx_per[e]
        cnt = len(ix)
        if cnt == 0:
            continue
        outt = results[e]["outt"].reshape(P, HT, C).transpose(1, 0, 2).reshape(H, C)
        np.add.at(out, tok[ix], outt[:, :cnt].T)
    return out


def _build(C, mode=None, repeat=1, loop_reps=0, body="full"):
    """Trace + compile the per-core bass program for padded count C.

    loop_reps > 0 wraps the body in a device-side For_i loop (timing only).
    body: "full" | "dma" (loads/stores only, no compute) | "compute"
    (loads hoisted out of the timing loop) - diagnostics only.
    """
    mode = mode or MODE
    f32 = mybir.dt.float32
    mdt = _mm_dt(mode)
    nc = bacc.Bacc("TRN2", target_bir_lowering=False, debug=False, num_devices=E)

    xt_d = nc.declare_dram_parameter("xt", [P, KH * C], mdt, isOutput=False)
    w13_d = nc.declare_dram_parameter("w13t", [P, KH * N], mdt, isOutput=False)
    w2_d = nc.declare_dram_parameter("w2t", [P, KI * H], mdt, isOutput=False)
    ag_d = nc.declare_dram_parameter("agt", [P, KH * LR], mdt, isOutput=False)
    ad_d = nc.declare_dram_parameter("adt", [P, KI * LR], mdt, isOutput=False)
    bg_d = nc.declare_dram_parameter("bgt", [LR, N], mdt, isOutput=False)
    bd_d = nc.declare_dram_parameter("bdt", [LR, H], mdt, isOutput=False)
    msk_d = nc.declare_dram_parameter("msk", [LR, C], f32, isOutput=False)
    wv_d = nc.declare_dram_parameter("wv", [P, C], f32, isOutput=False)
    odt = f32 if mybir.dt.size(mdt) == 4 else mdt
    out_d = nc.declare_dram_parameter("outt", [P, HT * C], odt, isOutput=True)

    # column blocks of at most 512 (PSUM free-dim limit for fp32)
    n_blk = (C + 511) // 512
    step = (C + n_blk - 1) // n_blk
    blks = [(b * step, min(C, (b + 1) * step)) for b in range(n_blk)]

    silu_fn = ACT_FN or mybir.ActivationFunctionType.Silu

    with tile.TileContext(nc) as tc:
        with ExitStack() as ctx:
            static = ctx.enter_context(tc.tile_pool(name="static", bufs=1))
            work = ctx.enter_context(tc.tile_pool(name="work", bufs=4))
            ptmp = ctx.enter_context(tc.tile_pool(name="ptmp", bufs=1, space="PSUM"))
            pc1g_pool = ctx.enter_context(tc.tile_pool(name="pc1g", bufs=3, space="PSUM"))
            pc1u_pool = ctx.enter_context(tc.tile_pool(name="pc1u", bufs=2, space="PSUM"))
            pc3 = ctx.enter_context(tc.tile_pool(name="pc3", bufs=2, space="PSUM"))

            # double-buffer input tiles for 2-byte modes so a loop/repeat
            # iteration's DMAs overlap the previous iteration's compute
            # (f32 tiles are too big to double-buffer in SBUF)
            sb = 2 if mybir.dt.size(mdt) == 2 else 1

            def make_tiles():
                t = {}
                t["xt"] = static.tile([P, KH * C], mdt, tag="xt", name="xt_sb", bufs=sb)
                t["w13"] = static.tile([P, KH * N], mdt, tag="w13", name="w13_sb", bufs=sb)
                t["w2"] = static.tile([P, KI * H], mdt, tag="w2", name="w2_sb", bufs=sb)
                t["ag"] = static.tile([P, KH * LR], mdt, tag="ag", name="ag_sb", bufs=sb)
                t["ad"] = static.tile([P, KI * LR], mdt, tag="ad", name="ad_sb", bufs=sb)
                t["bg"] = static.tile([LR, N], mdt, tag="bg", name="bg_sb", bufs=sb)
                t["bd"] = static.tile([LR, H], mdt, tag="bd", name="bd_sb", bufs=sb)
                t["msk"] = static.tile([LR, C], f32, tag="msk", name="msk_sb", bufs=sb)
                t["wv"] = static.tile([P, C], f32, tag="wv", name="wv_sb", bufs=sb)
                t["act"] = static.tile([P, KI * C], mdt, tag="act", name="act_sb")
                t["out"] = static.tile([P, HT * C], odt, tag="out", name="out_sb")
                t["xlg"] = static.tile([LR, C], mdt, tag="xlg", name="xlg_sb")
                t["xld"] = static.tile([LR, C], mdt, tag="xld", name="xld_sb")
                return t

            SW13 = KH * P   # free-dim cols per w13 slot (slot-major)
            SW2 = KI * P    # free-dim cols per w2 h-tile

            def emit_loads(t):
                # scalar ring: only the first few gate-critical loads (its
                # triggers retire before the first silu needs the engine).
                # sync ring: everything else in consumption order — once the
                # scalar ring drains (~t+13) a single HWDGE ring sustains
                # ~390 GB/s, so the tail (w2 etc) is not bandwidth-starved.
                sy, sc = nc.sync, nc.scalar

                def xk(ring, k0, k1):
                    ring.dma_start(t["xt"][:, k0 * C:k1 * C],
                                   xt_d[:, k0 * C:k1 * C])

                def ws(ring, s0, s1):
                    ring.dma_start(t["w13"][:, s0 * SW13:s1 * SW13],
                                   w13_d[:, s0 * SW13:s1 * SW13])

                xk(sy, 0, 1)
                sc.dma_start(t["w13"][:, :SW13 // 2], w13_d[:, :SW13 // 2])
                sc.dma_start(t["w13"][:, SW13 // 2:SW13], w13_d[:, SW13 // 2:SW13])
                xk(sy, 1, 5)
                ws(sc, 2, 3)
                xk(sy, 5, 8)
                ws(sc, 3, 4)
                ws(sy, 1, 2)
                sy.dma_start(t["ag"][:], ag_d[:])
                sy.dma_start(t["msk"][:], msk_d[:])
                sy.dma_start(t["bg"][:], bg_d[:])
                for s in range(4, NT, 2):
                    ws(sy, s, s + 2)
                sy.dma_start(t["ad"][:], ad_d[:])
                sy.dma_start(t["wv"][:], wv_d[:])
                sy.dma_start(t["bd"][:], bd_d[:])
                sy.dma_start(t["w2"][:, :HT // 2 * SW2], w2_d[:, :HT // 2 * SW2])
                sy.dma_start(t["w2"][:, HT // 2 * SW2:], w2_d[:, HT // 2 * SW2:])

            def emit_compute(t):
                # preload the ACT silu table at t~0 (otherwise a lazy 1.3us
                # ACT_TABLE_LOAD lands right before the first real silu and
                # stalls the pc1 PSUM recycle chain)
                warm = work.tile([P, 8], f32, tag="actwarm")
                nc.gpsimd.memset(warm[:], 0.0)
                nc.scalar.activation(warm[:], warm[:], silu_fn)

                # pre-warm the PE while the first DMAs are in flight: dummy
                # matmuls on a zeroed tile keep the HAM activity window busy
                # so the clock is at 2.4 GHz (not 1.2) when real data lands
                wcw = min(blks[0][1] - blks[0][0], 384 - P)
                wsrc = work.tile([P, 384], mdt, tag="mmwarm")
                nc.gpsimd.memset(wsrc[:], 0.0)
                wp = pc3.tile([P, wcw], f32, tag="c3")

                def fill(n):
                    for _ in range(n):
                        nc.tensor.matmul(wp[:], wsrc[:, :P],
                                         wsrc[:, P:P + wcw],
                                         start=True, stop=True)

                fill(22)

                for (c0, c1) in blks:
                    cw = c1 - c0

                    def xts(kt):
                        return t["xt"][:, kt * C + c0: kt * C + c1]

                    def acts(kt):
                        return t["act"][:, kt * C + c0: kt * C + c1]

                    def base13(c1t, slot):
                        for kt in range(KH):
                            off = slot * SW13 + kt * P
                            nc.tensor.matmul(
                                c1t[:], t["w13"][:, off:off + P], xts(kt),
                                start=(kt == 0), stop=False)

                    def bapply(c1t, jn):
                        nc.tensor.matmul(
                            c1t[:], t["bg"][:, jn * P:(jn + 1) * P],
                            t["xlg"][:, c0:c1], start=False, stop=True)

                    def drain(j, c1g, c1u):
                        silu_t = work.tile([P, cw], f32, tag="silu")
                        nc.scalar.activation(silu_t[:], c1g[:], silu_fn)
                        nc.vector.tensor_mul(acts(j), c1u[:], silu_t[:])

                    # pairs 0,1: base chains first so PE starts on the first
                    # w13 slot + xt chunk; the lora mid-product fills the
                    # window while slot2/slot3 are still in flight
                    held = []
                    c1g = pc1g_pool.tile([P, cw], f32, tag="c1g")
                    base13(c1g, 0)
                    c1u = pc1u_pool.tile([P, cw], f32, tag="c1u")
                    base13(c1u, 1)
                    held.append((0, c1g, c1u))

                    # gate_up lora mid-product
                    tmp_g = ptmp.tile([LR, cw], f32, tag="tmp")
                    for kt in range(KH):
                        nc.tensor.matmul(
                            tmp_g[:], t["ag"][:, kt * LR:(kt + 1) * LR], xts(kt),
                            start=(kt == 0), stop=(kt == KH - 1))

                    c1g = pc1g_pool.tile([P, cw], f32, tag="c1g")
                    base13(c1g, 2)
                    c1u = pc1u_pool.tile([P, cw], f32, tag="c1u")
                    base13(c1u, 3)
                    held.append((1, c1g, c1u))

                    nc.vector.tensor_mul(t["xlg"][:, c0:c1], tmp_g[:],
                                         t["msk"][:, c0:c1])

                    for (j, c1g, c1u) in held:
                        bapply(c1g, j)
                        bapply(c1u, j + KI)
                        drain(j, c1g, c1u)

                    for j in range(2, KI):
                        c1g = pc1g_pool.tile([P, cw], f32, tag="c1g")
                        base13(c1g, 2 * j)
                        bapply(c1g, j)
                        c1u = pc1u_pool.tile([P, cw], f32, tag="c1u")
                        base13(c1u, 2 * j + 1)
                        bapply(c1u, j + KI)
                        drain(j, c1g, c1u)

                    # down lora mid-product
                    tmp_d = ptmp.tile([LR, cw], f32, tag="tmp")
                    for kt in range(KI):
                        nc.tensor.matmul(
                            tmp_d[:], t["ad"][:, kt * LR:(kt + 1) * LR], acts(kt),
                            start=(kt == 0), stop=(kt == KI - 1))
                    nc.vector.tensor_mul(t["xld"][:, c0:c1], tmp_d[:],
                                         t["msk"][:, c0:c1])

                    # down GEMM + lora; routing weight applied at the output
                    for h in range(HT):
                        c3t = pc3.tile([P, cw], f32, tag="c3")
                        for kt in range(KI):
                            off = h * SW2 + kt * P
                            nc.tensor.matmul(
                                c3t[:], t["w2"][:, off:off + P], acts(kt),
                                start=(kt == 0), stop=False)
                        nc.tensor.matmul(
                            c3t[:], t["bd"][:, h * P:(h + 1) * P],
                            t["xld"][:, c0:c1], start=False, stop=True)
                        nc.vector.tensor_mul(
                            t["out"][:, h * C + c0: h * C + c1], c3t[:],
                            t["wv"][:, c0:c1])
                        # stores ride the scalar HWDGE ring, which is idle
                        # by the time the down phase produces output tiles
                        if c0 == 0 and c1 == C and (h % 2 == 1 or h >= HT - 2):
                            lo = (h - 1) * C if (h % 2 == 1 and h < HT - 1) else h * C
                            nc.scalar.dma_start(
                                out_d[:, lo:(h + 1) * C],
                                t["out"][:, lo:(h + 1) * C])
            def emit_block_stores(t):
                # fallback for multi-block shapes (C > 512): bulk store
                if not (len(blks) == 1 and blks[0] == (0, C)):
                    half = HT * C // 2
                    nc.sync.dma_start(out_d[:, :half], t["out"][:, :half])
                    nc.scalar.dma_start(out_d[:, half:], t["out"][:, half:])

            hoisted = None
            if body == "compute":
                hoisted = make_tiles()
                emit_loads(hoisted)

            loop_ctx = None
            if loop_reps > 0:
                loop_ctx = tc.For_i(
                    0, loop_reps, 1,
                    hint_engines=(mybir.EngineType.PE, mybir.EngineType.DVE,
                                  mybir.EngineType.Activation,
                                  mybir.EngineType.SP))
                loop_ctx.__enter__()

            for _rep in range(repeat):
                t = hoisted if hoisted is not None else make_tiles()
                if body == "full":
                    emit_loads(t)
                    emit_compute(t)
                elif body == "dma":
                    emit_loads(t)
                elif body == "compute":
                    emit_compute(t)
                if body != "dma":
                    emit_block_stores(t)

            if loop_ctx is not None:
                loop_ctx.__exit__(None, None, None)

    nc.compile()
    return nc


def _get_nc(C, mode=None, repeat=1, loop_reps=0, body="full"):
    mode = mode or MODE
    key = (C, mode, repeat, loop_reps, body)
    if key not in _CACHE:
        _CACHE[key] = _build(C, mode, repeat, loop_reps, body)
    return _CACHE[key]


def kernel(hidden_states, topk_weights, w13, w2, gate_up_lora_a,
           gate_up_lora_b, down_lora_a, down_lora_b, scalings,
           topk_ids, lora_indices, mode=None):
    in_maps, idx_per, tok, C = _prep_in_maps(
        hidden_states, topk_weights, w13, w2, gate_up_lora_a,
        gate_up_lora_b, down_lora_a, down_lora_b, scalings,
        topk_ids, lora_indices, mode=mode)
    nc = _get_nc(C, mode)
    res = run_bass_kernel_spmd(nc, in_maps, list(range(E)))
    out = _combine(res.results, idx_per, tok, C)
    return out.astype(np.asarray(hidden_states).dtype)



# revision 24
# speedup vs baseline: 1.0529x; 1.0290x over previous
"""Fused MoE + per-expert LoRA, expert-parallel across 8 TRN2 NeuronCores.

Strategy (sharding hint: expert-parallel):
  - Host dispatches the T*K routed (token, expert) pairs: core e gets the
    tokens routed to expert e, gathered + transposed to [H, C] (C = padded
    max per-expert count), plus expert e's w13/w2 (slot-major tile layout,
    so every per-slot DMA is contiguous) and the 4 LoRA adapters' A/B for
    expert e, pre-transposed on host into the SBUF layouts the kernel uses.
  - LoRA is fused into the base GEMMs: the 4 adapters' A matrices are
    concatenated to [64, H] so one extra matmul chain produces all
    mid-products; a [64, C] mask (scaling[l] where the pair's adapter == l,
    else 0) selects the right adapter per column; the concatenated B
    matrices accumulate into the same PSUM tile as the base GEMM. B columns
    are math-ordered (gate j at col j*P, up j at (j+KI)*P) while w13 slots
    are gate/up-interleaved via W13_PERM.
  - Routing weight is applied at the output tiles (exact: column scaling
    commutes through the down GEMM and its LoRA delta).
  - Everything runs in f16 (rel err ~7e-4): halves HBM traffic vs f32 and
    moves the kernel from DMA-bound to PE-bound (~33us of matmul streaming
    at ~141ns per 280-col matmul).
  - Schedule: the scalar HWDGE ring carries only the 4 earliest loads (it
    must be free for silu after ~t+14); the sync ring carries the rest in
    exact PE-consumption order (a single ring sustains ~390 GB/s once the
    other drains). Output tiles store via the scalar ring during the down
    phase. 22 dummy matmuls on a zeroed tile pre-warm the PE clock (HAM
    K=4/8 -> 8/8) while the first DMAs are in flight, and the LoRA
    mid-chain is emitted between the pair-0/1 base chains as gap filler.
  - Host scatter-adds each core's [H, C] f16 output back over top_k.
"""

import numpy as np
from contextlib import ExitStack

import concourse.bass as bass
import concourse.tile as tile
from concourse import bacc, mybir
from concourse.bass_utils import run_bass_kernel_spmd

T, H, I, E, K, L, R = 1024, 1024, 1024, 8, 2, 4, 16
N = 2 * I
P = 128
KH = H // P    # k-tiles over H (gate_up contraction)
KI = I // P    # k-tiles over I (down contraction)
NT = N // P    # n-tiles of gate_up output
HT = H // P    # h-tiles of down output
LR = L * R     # concatenated lora rank

# w13 SBUF slot s holds n-tile W13_PERM[s]: gate/up tiles interleaved so the
# compute loop consumes slots sequentially (pair j = slots 2j, 2j+1)
W13_PERM = [t for j in range(NT // 2) for t in (j, j + NT // 2)]

_CACHE: dict = {}

# f16 halves HBM traffic vs f32 (w13+w2 dominate); matmul streams at the
# same 1 col/cycle either way, so this moves the kernel from DMA-bound
# (~46us of weight DMA) to PE-bound (~29us). randn-scale data sits well
# inside f16 range; rel err ~1e-3 << the 2e-2 gate.
MODE = "f16"
ACT_FN = None  # debug hook: sim_check overrides (CoreSim lacks Silu)


def _round_up(x, m):
    return ((x + m - 1) // m) * m


def _np_dt(mode):
    if mode == "bf16":
        import ml_dtypes
        return np.dtype(ml_dtypes.bfloat16)
    if mode == "f16":
        return np.dtype(np.float16)
    return np.dtype(np.float32)


def _mm_dt(mode):
    return {"f32": mybir.dt.float32,
            "f32r": mybir.dt.float32r,
            "bf16": mybir.dt.bfloat16,
            "f16": mybir.dt.float16}[mode]


def _prep_in_maps(hidden_states, topk_weights, w13, w2, gate_up_lora_a,
                  gate_up_lora_b, down_lora_a, down_lora_b, scalings,
                  topk_ids, lora_indices, mode=None):
    """Host-side dispatch: returns (in_maps, idx_per_expert, tok, C)."""
    mode = mode or MODE
    ndt = _np_dt(mode)
    hidden_states = np.asarray(hidden_states, dtype=np.float32)
    topk_weights = np.asarray(topk_weights, dtype=np.float32)
    w13 = np.asarray(w13, dtype=np.float32)
    w2 = np.asarray(w2, dtype=np.float32)
    gua = np.asarray(gate_up_lora_a, dtype=np.float32)
    gub = np.asarray(gate_up_lora_b, dtype=np.float32)
    dla = np.asarray(down_lora_a, dtype=np.float32)
    dlb = np.asarray(down_lora_b, dtype=np.float32)
    scalings = np.asarray(scalings, dtype=np.float32)
    topk_ids = np.asarray(topk_ids)
    lora_indices = np.asarray(lora_indices)

    e_flat = topk_ids.reshape(-1).astype(np.int64)
    l_flat = np.repeat(lora_indices, K).astype(np.int64)
    w_flat = topk_weights.reshape(-1).astype(np.float32)
    tok = np.arange(T * K) // K

    idx_per = [np.nonzero(e_flat == e)[0] for e in range(E)]
    maxc = max(1, max(len(ix) for ix in idx_per))
    C = max(128, _round_up(maxc, 4))

    in_maps = []
    for e in range(E):
        ix = idx_per[e]
        cnt = len(ix)

        xg = np.zeros((C, H), np.float32)
        xg[:cnt] = hidden_states[tok[ix]]
        xt = np.ascontiguousarray(
            xg.T.reshape(KH, P, C).transpose(1, 0, 2).reshape(P, KH * C)).astype(ndt)

        w13t = w13[e].T  # [H, N]
        # slot-major layout: slot s (gate/up interleaved via W13_PERM) is a
        # contiguous [P, KH*P] span -> per-slot DMAs are contiguous
        w13_tiles = w13t.reshape(KH, P, NT, P)[:, :, W13_PERM]
        w13_hbm = np.ascontiguousarray(
            w13_tiles.transpose(1, 2, 0, 3).reshape(P, KH * N)).astype(ndt)

        w2t = w2[e].T  # [I, H]
        w2_hbm = np.ascontiguousarray(
            w2t.reshape(KI, P, HT, P)
            .transpose(1, 2, 0, 3).reshape(P, KI * H)).astype(ndt)

        ag = np.ascontiguousarray(
            gua[:, e].reshape(LR, H).T.reshape(KH, P, LR)
            .transpose(1, 0, 2).reshape(P, KH * LR)).astype(ndt)
        ad = np.ascontiguousarray(
            dla[:, e].reshape(LR, I).T.reshape(KI, P, LR)
            .transpose(1, 0, 2).reshape(P, KI * LR)).astype(ndt)
        bg = np.ascontiguousarray(
            gub[:, e].transpose(0, 2, 1).reshape(LR, N)).astype(ndt)
        bd = np.ascontiguousarray(
            dlb[:, e].transpose(0, 2, 1).reshape(LR, H)).astype(ndt)

        msk = np.zeros((LR, C), np.float32)
        if cnt:
            lv = l_flat[ix]
            m_small = (lv[None, :] == np.arange(L)[:, None]) * scalings[:, None]
            msk[:, :cnt] = np.repeat(m_small.astype(np.float32), R, axis=0)

        wv = np.zeros((P, C), np.float32)
        if cnt:
            wv[:, :cnt] = w_flat[ix][None, :]

        in_maps.append({
            "xt": xt, "w13t": w13_hbm, "w2t": w2_hbm,
            "agt": ag, "adt": ad, "bgt": bg, "bdt": bd,
            "msk": msk, "wv": wv,
        })
    return in_maps, idx_per, tok, C


def _combine(results, idx_per, tok, C):
    out = np.zeros((T, H), np.float32)
    for e in range(E):
        ix = idx_per[e]
        cnt = len(ix)
        if cnt == 0:
            continue
        outt = results[e]["outt"].reshape(P, HT, C).transpose(1, 0, 2).reshape(H, C)
        np.add.at(out, tok[ix], outt[:, :cnt].T)
    return out


def _build(C, mode=None, repeat=1, loop_reps=0, body="full"):
    """Trace + compile the per-core bass program for padded count C.

    loop_reps > 0 wraps the body in a device-side For_i loop (timing only).
    body: "full" | "dma" (loads/stores only, no compute) | "compute"
    (loads hoisted out of the timing loop) - diagnostics only.
    """
    mode = mode or MODE
    f32 = mybir.dt.float32
    mdt = _mm_dt(mode)
    nc = bacc.Bacc("TRN2", target_bir_lowering=False, debug=False, num_devices=E)

    xt_d = nc.declare_dram_parameter("xt", [P, KH * C], mdt, isOutput=False)
    w13_d = nc.declare_dram_parameter("w13t", [P, KH * N], mdt, isOutput=False)
    w2_d = nc.declare_dram_parameter("w2t", [P, KI * H], mdt, isOutput=False)
    ag_d = nc.declare_dram_parameter("agt", [P, KH * LR], mdt, isOutput=False)
    ad_d = nc.declare_dram_parameter("adt", [P, KI * LR], mdt, isOutput=False)
    bg_d = nc.declare_dram_parameter("bgt", [LR, N], mdt, isOutput=False)
    bd_d = nc.declare_dram_parameter("bdt", [LR, H], mdt, isOutput=False)
    msk_d = nc.declare_dram_parameter("msk", [LR, C], f32, isOutput=False)
    wv_d = nc.declare_dram_parameter("wv", [P, C], f32, isOutput=False)
    odt = f32 if mybir.dt.size(mdt) == 4 else mdt
    out_d = nc.declare_dram_parameter("outt", [P, HT * C], odt, isOutput=True)

    # column blocks of at most 512 (PSUM free-dim limit for fp32)
    n_blk = (C + 511) // 512
    step = (C + n_blk - 1) // n_blk
    blks = [(b * step, min(C, (b + 1) * step)) for b in range(n_blk)]

    silu_fn = ACT_FN or mybir.ActivationFunctionType.Silu

    with tile.TileContext(nc) as tc:
        with ExitStack() as ctx:
            static = ctx.enter_context(tc.tile_pool(name="static", bufs=1))
            work = ctx.enter_context(tc.tile_pool(name="work", bufs=4))
            ptmp = ctx.enter_context(tc.tile_pool(name="ptmp", bufs=1, space="PSUM"))
            pc1g_pool = ctx.enter_context(tc.tile_pool(name="pc1g", bufs=3, space="PSUM"))
            pc1u_pool = ctx.enter_context(tc.tile_pool(name="pc1u", bufs=2, space="PSUM"))
            pc3 = ctx.enter_context(tc.tile_pool(name="pc3", bufs=2, space="PSUM"))

            # double-buffer input tiles for 2-byte modes so a loop/repeat
            # iteration's DMAs overlap the previous iteration's compute
            # (f32 tiles are too big to double-buffer in SBUF)
            sb = 2 if mybir.dt.size(mdt) == 2 else 1

            def make_tiles():
                t = {}
                t["xt"] = static.tile([P, KH * C], mdt, tag="xt", name="xt_sb", bufs=sb)
                t["w13"] = static.tile([P, KH * N], mdt, tag="w13", name="w13_sb", bufs=sb)
                t["w2"] = static.tile([P, KI * H], mdt, tag="w2", name="w2_sb", bufs=sb)
                t["ag"] = static.tile([P, KH * LR], mdt, tag="ag", name="ag_sb", bufs=sb)
                t["ad"] = static.tile([P, KI * LR], mdt, tag="ad", name="ad_sb", bufs=sb)
                t["bg"] = static.tile([LR, N], mdt, tag="bg", name="bg_sb", bufs=sb)
                t["bd"] = static.tile([LR, H], mdt, tag="bd", name="bd_sb", bufs=sb)
                t["msk"] = static.tile([LR, C], f32, tag="msk", name="msk_sb", bufs=sb)
                t["wv"] = static.tile([P, C], f32, tag="wv", name="wv_sb", bufs=sb)
                t["act"] = static.tile([P, KI * C], mdt, tag="act", name="act_sb")
                t["out"] = static.tile([P, HT * C], odt, tag="out", name="out_sb")
                t["xlg"] = static.tile([LR, C], mdt, tag="xlg", name="xlg_sb")
                t["xld"] = static.tile([LR, C], mdt, tag="xld", name="xld_sb")
                return t

            SW13 = KH * P   # free-dim cols per w13 slot (slot-major)
            SW2 = KI * P    # free-dim cols per w2 h-tile

            def emit_loads(t):
                # scalar ring: only the first few gate-critical loads (its
                # triggers retire before the first silu needs the engine).
                # sync ring: everything else in consumption order — once the
                # scalar ring drains (~t+13) a single HWDGE ring sustains
                # ~390 GB/s, so the tail (w2 etc) is not bandwidth-starved.
                sy, sc = nc.sync, nc.scalar

                def xk(ring, k0, k1):
                    ring.dma_start(t["xt"][:, k0 * C:k1 * C],
                                   xt_d[:, k0 * C:k1 * C])

                def ws(ring, s0, s1):
                    ring.dma_start(t["w13"][:, s0 * SW13:s1 * SW13],
                                   w13_d[:, s0 * SW13:s1 * SW13])

                xk(sy, 0, 1)
                sc.dma_start(t["ag"][:], ag_d[:])
                xk(sy, 1, 5)
                sc.dma_start(t["w13"][:, :SW13], w13_d[:, :SW13])
                xk(sy, 5, 8)
                ws(sc, 2, 3)
                ws(sy, 1, 2)
                ws(sc, 3, 4)
                sy.dma_start(t["msk"][:], msk_d[:])
                sy.dma_start(t["bg"][:], bg_d[:])
                for s in range(4, NT, 2):
                    ws(sy, s, s + 2)
                sy.dma_start(t["ad"][:], ad_d[:])
                sy.dma_start(t["wv"][:], wv_d[:])
                sy.dma_start(t["bd"][:], bd_d[:])
                sy.dma_start(t["w2"][:, :HT // 2 * SW2], w2_d[:, :HT // 2 * SW2])
                sy.dma_start(t["w2"][:, HT // 2 * SW2:], w2_d[:, HT // 2 * SW2:])

            def emit_compute(t):
                # preload the ACT silu table at t~0 (otherwise a lazy 1.3us
                # ACT_TABLE_LOAD lands right before the first real silu and
                # stalls the pc1 PSUM recycle chain)
                warm = work.tile([P, 8], f32, tag="actwarm")
                nc.gpsimd.memset(warm[:], 0.0)
                nc.scalar.activation(warm[:], warm[:], silu_fn)

                # pre-warm the PE while the first DMAs are in flight: dummy
                # matmuls on a zeroed tile keep the HAM activity window busy
                # so the clock is at 2.4 GHz (not 1.2) when real data lands
                wcw = min(blks[0][1] - blks[0][0], 384 - P)
                wsrc = work.tile([P, 384], mdt, tag="mmwarm")
                nc.gpsimd.memset(wsrc[:], 0.0)
                wp = pc3.tile([P, wcw], f32, tag="c3")

                def fill(n):
                    for _ in range(n):
                        nc.tensor.matmul(wp[:], wsrc[:, :P],
                                         wsrc[:, P:P + wcw],
                                         start=True, stop=True)

                fill(22)

                for (c0, c1) in blks:
                    cw = c1 - c0

                    def xts(kt):
                        return t["xt"][:, kt * C + c0: kt * C + c1]

                    def acts(kt):
                        return t["act"][:, kt * C + c0: kt * C + c1]

                    def base13(c1t, slot):
                        for kt in range(KH):
                            off = slot * SW13 + kt * P
                            nc.tensor.matmul(
                                c1t[:], t["w13"][:, off:off + P], xts(kt),
                                start=(kt == 0), stop=False)

                    def bapply(c1t, jn):
                        nc.tensor.matmul(
                            c1t[:], t["bg"][:, jn * P:(jn + 1) * P],
                            t["xlg"][:, c0:c1], start=False, stop=True)

                    def drain(j, c1g, c1u):
                        silu_t = work.tile([P, cw], f32, tag="silu")
                        nc.scalar.activation(silu_t[:], c1g[:], silu_fn)
                        nc.vector.tensor_mul(acts(j), c1u[:], silu_t[:])

                    # gate_up lora mid-product runs first among the real
                    # chains: it needs only ag (125 KB) + xt, so each w13
                    # slot gets an extra ~1.1us of DMA time before its chain
                    tmp_g = ptmp.tile([LR, cw], f32, tag="tmp")
                    for kt in range(KH):
                        nc.tensor.matmul(
                            tmp_g[:], t["ag"][:, kt * LR:(kt + 1) * LR], xts(kt),
                            start=(kt == 0), stop=(kt == KH - 1))

                    held = []
                    c1g = pc1g_pool.tile([P, cw], f32, tag="c1g")
                    base13(c1g, 0)
                    c1u = pc1u_pool.tile([P, cw], f32, tag="c1u")
                    base13(c1u, 1)
                    held.append((0, c1g, c1u))

                    nc.vector.tensor_mul(t["xlg"][:, c0:c1], tmp_g[:],
                                         t["msk"][:, c0:c1])

                    c1g = pc1g_pool.tile([P, cw], f32, tag="c1g")
                    base13(c1g, 2)
                    c1u = pc1u_pool.tile([P, cw], f32, tag="c1u")
                    base13(c1u, 3)
                    held.append((1, c1g, c1u))

                    for (j, c1g, c1u) in held:
                        bapply(c1g, j)
                        bapply(c1u, j + KI)
                        drain(j, c1g, c1u)

                    for j in range(2, KI):
                        c1g = pc1g_pool.tile([P, cw], f32, tag="c1g")
                        base13(c1g, 2 * j)
                        bapply(c1g, j)
                        c1u = pc1u_pool.tile([P, cw], f32, tag="c1u")
                        base13(c1u, 2 * j + 1)
                        bapply(c1u, j + KI)
                        drain(j, c1g, c1u)

                    # down lora mid-product
                    tmp_d = ptmp.tile([LR, cw], f32, tag="tmp")
                    for kt in range(KI):
                        nc.tensor.matmul(
                            tmp_d[:], t["ad"][:, kt * LR:(kt + 1) * LR], acts(kt),
                            start=(kt == 0), stop=(kt == KI - 1))
                    nc.vector.tensor_mul(t["xld"][:, c0:c1], tmp_d[:],
                                         t["msk"][:, c0:c1])

                    # down GEMM + lora; routing weight applied at the output
                    for h in range(HT):
                        c3t = pc3.tile([P, cw], f32, tag="c3")
                        for kt in range(KI):
                            off = h * SW2 + kt * P
                            nc.tensor.matmul(
                                c3t[:], t["w2"][:, off:off + P], acts(kt),
                                start=(kt == 0), stop=False)
                        nc.tensor.matmul(
                            c3t[:], t["bd"][:, h * P:(h + 1) * P],
                            t["xld"][:, c0:c1], start=False, stop=True)
                        nc.vector.tensor_mul(
                            t["out"][:, h * C + c0: h * C + c1], c3t[:],
                            t["wv"][:, c0:c1])
                        # stores ride the scalar HWDGE ring, which is idle
                        # by the time the down phase produces output tiles
                        if c0 == 0 and c1 == C and (h % 2 == 1 or h >= HT - 2):
                            lo = (h - 1) * C if (h % 2 == 1 and h < HT - 1) else h * C
                            nc.scalar.dma_start(
                                out_d[:, lo:(h + 1) * C],
                                t["out"][:, lo:(h + 1) * C])
            def emit_block_stores(t):
                # fallback for multi-block shapes (C > 512): bulk store
                if not (len(blks) == 1 and blks[0] == (0, C)):
                    half = HT * C // 2
                    nc.sync.dma_start(out_d[:, :half], t["out"][:, :half])
                    nc.scalar.dma_start(out_d[:, half:], t["out"][:, half:])

            hoisted = None
            if body == "compute":
                hoisted = make_tiles()
                emit_loads(hoisted)

            loop_ctx = None
            if loop_reps > 0:
                loop_ctx = tc.For_i(
                    0, loop_reps, 1,
                    hint_engines=(mybir.EngineType.PE, mybir.EngineType.DVE,
                                  mybir.EngineType.Activation,
                                  mybir.EngineType.SP))
                loop_ctx.__enter__()

            for _rep in range(repeat):
                t = hoisted if hoisted is not None else make_tiles()
                if body == "full":
                    emit_loads(t)
                    emit_compute(t)
                elif body == "dma":
                    emit_loads(t)
                elif body == "compute":
                    emit_compute(t)
                if body != "dma":
                    emit_block_stores(t)

            if loop_ctx is not None:
                loop_ctx.__exit__(None, None, None)

    nc.compile()
    return nc


def _get_nc(C, mode=None, repeat=1, loop_reps=0, body="full"):
    mode = mode or MODE
    key = (C, mode, repeat, loop_reps, body)
    if key not in _CACHE:
        _CACHE[key] = _build(C, mode, repeat, loop_reps, body)
    return _CACHE[key]


def kernel(hidden_states, topk_weights, w13, w2, gate_up_lora_a,
           gate_up_lora_b, down_lora_a, down_lora_b, scalings,
           topk_ids, lora_indices, mode=None):
    in_maps, idx_per, tok, C = _prep_in_maps(
        hidden_states, topk_weights, w13, w2, gate_up_lora_a,
        gate_up_lora_b, down_lora_a, down_lora_b, scalings,
        topk_ids, lora_indices, mode=mode)
    nc = _get_nc(C, mode)
    res = run_bass_kernel_spmd(nc, in_maps, list(range(E)))
    out = _combine(res.results, idx_per, tok, C)
    return out.astype(np.asarray(hidden_states).dtype)

